# revision 1
# baseline (speedup 1.0000x reference)
"""ARD-RBF covariance kernel for Trainium2 (Bass/Tile), 8-core row-parallel.

Math (matches the reference):
    s  = exp(-weights[:, 0])                      # (D,) inverse lengthscales
    Us = U * s ; Vs = V * s
    sq[i, j] = ||Us_i||^2 + ||Vs_j||^2 - 2 Us_i . Vs_j
    K[i, j]  = exp(2*sn) * exp(-0.5 * max(sq, 0))

Device strategy (per core, rows sharded 8 ways):
    One augmented matmul computes sq directly in PSUM:
      lhsT (K=18 x 128) rows: [-2*s*U^T block ; ||Us||^2 row ; ones row]
      rhs  (K=18 x 512) rows: [ s*V^T        ; ones row     ; ||Vs||^2 row]
    Then a single ScalarE activation per tile computes
      out = Exp(-0.5 * psum + 2*sn)   (PSUM -> SBUF), and DMA writes out.
    Row norms are computed on-device via ones-vector matmuls.

The (8192, 8192) f32 output (256 MB) makes this memory-bound on the
HBM write (~90 us/core at ~358 GB/s); PE/ACT work is overlapped.
"""

import numpy as np

import concourse.bacc as bacc
import concourse.bass as bass  # noqa: F401  (AP helpers)
import concourse.mybir as mybir
import concourse.tile as tile

N, M, D = 8192, 8192, 16
N_CORES = 8
ROWS = N // N_CORES  # 1024 rows of U per core
P = 128              # output partitions per row block
FREE = 512           # matmul moving free dim (one PSUM bank of f32)
QUAD = 2048          # ACT chunk: 4 banks
K = D + 2            # augmented contraction dim

F32 = mybir.dt.float32
F32R = mybir.dt.float32r
AF = mybir.ActivationFunctionType


def build_program(rows=ROWS, m_cols=M, repeats=1):
    """Build the per-core Bass program. rows/m_cols shrinkable for sim."""
    rb = rows // P
    nq = m_cols // QUAD

    nc = bacc.Bacc()
    ut = nc.declare_dram_parameter("ut", [D, rows], F32, isOutput=False)
    vt = nc.declare_dram_parameter("vt", [D, m_cols], F32, isOutput=False)
    w = nc.declare_dram_parameter("w", [D, 1], F32, isOutput=False)
    sn = nc.declare_dram_parameter("sn", [1, 1], F32, isOutput=False)
    out = nc.declare_dram_parameter("out", [rows, m_cols], F32, isOutput=True)

    with tile.TileContext(nc) as tc:
        with (
            tc.tile_pool(name="singles", bufs=1) as singles,
            tc.tile_pool(name="scratch", bufs=2) as scratch,
            tc.tile_pool(name="psum_pool", bufs=2, space="PSUM") as psum_pool,
            tc.tile_pool(name="obuf_pool", bufs=3) as obuf_pool,
        ):
            # --- scale factors -------------------------------------------
            wt = singles.tile([D, 1], F32)
            nc.sync.dma_start(wt[:], w[:])
            s_t = singles.tile([D, 1], F32)
            nc.scalar.activation(s_t[:], wt[:], AF.Exp, scale=-1.0)  # s = exp(-w)
            s2_t = singles.tile([D, 1], F32)
            nc.scalar.mul(s2_t[:], s_t[:], -2.0)                     # -2s

            snb = singles.tile([P, 1], F32)
            nc.gpsimd.dma_start(snb[:], sn[:].to_broadcast((P, 1)))
            bias2 = singles.tile([P, 1], F32)
            nc.scalar.mul(bias2[:], snb[:], 2.0)                     # 2*sn

            ones16 = singles.tile([D, 1], F32)
            nc.vector.memset(ones16[:], 1.0)
            quart16 = singles.tile([D, 1], F32)
            nc.vector.memset(quart16[:], 0.25)

            # Compute-engine SBUF APs must start at partition 0/32/64/96, so
            # the augmented rows (16, 17) are built in partition-0 scratch
            # tiles and DMA'd into place (DMA has no partition restriction).
            onesrow = singles.tile([1, QUAD], F32)
            nc.vector.memset(onesrow[:], 1.0)

            # --- lhsT: L = [-2 s U^T ; u2 ; 1] ---------------------------
            # L/R carry 4 copies of the K=18 operand at partitions 0/32/64/96
            # so 4 matmuls can run concurrently in the PE's four 32-row
            # groups (tile_position row tiling) — hides the fp32 LDWEIGHTS
            # and 2-pass matmul cost behind concurrent streaming.
            L = singles.tile([3 * 32 + K, rows], F32)
            nc.sync.dma_start(L[0:D, :], ut[:])
            # tensor_tensor with a broadcast AP rather than tensor_scalar:
            # TensorScalarPtr only has one sync-wait slot in the ISA.
            nc.vector.tensor_mul(L[0:D, :], L[0:D, :], s2_t.to_broadcast((D, rows)))
            for c in range(rows // QUAD + (1 if rows % QUAD else 0)):
                w_ = min(QUAD, rows - c * QUAD)
                nc.sync.dma_start(
                    L[D + 1 : D + 2, c * QUAD : c * QUAD + w_], onesrow[:, :w_]
                )
            qU = singles.tile([D, rows], F32)
            nc.vector.tensor_mul(qU[:], L[0:D, :], L[0:D, :])        # 4 s^2 U^2
            u2row = singles.tile([1, rows], F32)
            for c in range(rows // FREE):
                ps = psum_pool.tile([P, QUAD], F32, tag="ps", name="ps")
                nc.tensor.matmul(
                    ps[0:1, 0:FREE], quart16[:], qU[:, c * FREE : (c + 1) * FREE],
                    start=True, stop=True,
                )
                nc.vector.tensor_copy(
                    u2row[:, c * FREE : (c + 1) * FREE], ps[0:1, 0:FREE]
                )
            nc.sync.dma_start(L[D : D + 1, :], u2row[:])
            for g in range(1, 4):
                nc.sync.dma_start(L[32 * g : 32 * g + K, :], L[0:K, :])

            # --- rhs: R = [s V^T ; 1 ; v2], built per 2048-col group -----
            R = singles.tile([3 * 32 + K, m_cols], F32)
            nc.sync.dma_start(R[0:D, :], vt[:])
            for g in range(m_cols // QUAD):
                gsl = slice(g * QUAD, (g + 1) * QUAD)
                nc.vector.tensor_mul(
                    R[0:D, gsl], R[0:D, gsl], s_t.to_broadcast((D, QUAD))
                )
                nc.sync.dma_start(R[D : D + 1, gsl], onesrow[:])
                qvg = scratch.tile([D, QUAD], F32, tag="qvg", name="qvg")
                nc.vector.tensor_mul(qvg[:], R[0:D, gsl], R[0:D, gsl])  # s^2 V^2
                vrow = scratch.tile([1, QUAD], F32, tag="vrow", name="vrow")
                for c in range(QUAD // FREE):
                    ps = psum_pool.tile([P, QUAD], F32, tag="ps", name="ps")
                    nc.tensor.matmul(
                        ps[0:1, 0:FREE], ones16[:], qvg[:, c * FREE : (c + 1) * FREE],
                        start=True, stop=True,
                    )
                    nc.vector.tensor_copy(
                        vrow[:, c * FREE : (c + 1) * FREE], ps[0:1, 0:FREE]
                    )
                nc.sync.dma_start(R[D + 1 : D + 2, gsl], vrow[:])
            for g in range(1, 4):
                nc.sync.dma_start(R[32 * g : 32 * g + K, :], R[0:K, :])

            # --- main loop ----------------------------------------------
            for _rep in range(repeats):
                for m in range(rb):
                    ob = obuf_pool.tile([P, m_cols], F32, tag="ob", name="ob")
                    for q in range(nq):
                        ps = psum_pool.tile([P, QUAD], F32, tag="ps", name="ps")
                        for k in range(QUAD // FREE):
                            n = q * (QUAD // FREE) + k
                            nc.tensor.matmul(
                                ps[:, k * FREE : (k + 1) * FREE],
                                L[32 * k : 32 * k + K, m * P : (m + 1) * P],
                                R[32 * k : 32 * k + K, n * FREE : (n + 1) * FREE],
                                start=True, stop=True,
                                tile_position=(32 * k, 0),
                            )
                        nc.scalar.activation(
                            ob[:, q * QUAD : (q + 1) * QUAD], ps[:],
                            AF.Exp, bias=bias2[:], scale=-0.5,
                        )
                        # store each 1MB quad as soon as its ACT lands so the
                        # DMA stream overlaps the ACT stream
                        nc.sync.dma_start(
                            out[m * P : (m + 1) * P, q * QUAD : (q + 1) * QUAD],
                            ob[:, q * QUAD : (q + 1) * QUAD],
                        )

    nc.compile()  # bacc lowering: splits multi-waits, reg alloc, etc.
    return nc


_PROGRAM_CACHE = {}


def get_program(rows=ROWS, m_cols=M, repeats=1):
    key = (rows, m_cols, repeats)
    if key not in _PROGRAM_CACHE:
        _PROGRAM_CACHE[key] = build_program(rows, m_cols, repeats)
    return _PROGRAM_CACHE[key]


def make_in_maps(U, V, weights, sn):
    U = np.ascontiguousarray(np.asarray(U, dtype=np.float32))
    V = np.ascontiguousarray(np.asarray(V, dtype=np.float32))
    w = np.ascontiguousarray(np.asarray(weights, dtype=np.float32).reshape(D, 1))
    snr = np.asarray(sn, dtype=np.float32).reshape(1, 1)
    vt = np.ascontiguousarray(V.T)
    in_maps = []
    for c in range(N_CORES):
        ut = np.ascontiguousarray(U[c * ROWS : (c + 1) * ROWS].T)
        in_maps.append({"ut": ut, "vt": vt, "w": w, "sn": snr})
    return in_maps


def kernel(U, V, weights, sn):
    from concourse.bass_utils import run_bass_kernel_spmd

    nc = get_program()
    in_maps = make_in_maps(U, V, weights, sn)
    res = run_bass_kernel_spmd(nc, in_maps, core_ids=list(range(N_CORES)))
    return np.concatenate([r["out"] for r in res.results], axis=0)



# revision 3
# speedup vs baseline: 3.1636x; 3.1636x over previous
"""ARD-RBF covariance kernel for Trainium2 (Bass/Tile), 8-core row-parallel.

Math (matches the reference):
    s  = exp(-weights[:, 0])                      # (D,) inverse lengthscales
    Us = U * s ; Vs = V * s
    sq[i, j] = ||Us_i||^2 + ||Vs_j||^2 - 2 Us_i . Vs_j
    K[i, j]  = exp(2*sn) * exp(-0.5 * max(sq, 0))

Device strategy (per core, rows sharded 8 ways):
    The augmented operands are built on the HOST (U/V are only 8192x16, so
    prep is trivial numpy) and DMA'd in ready-to-use:
      L (114 x rows): 4 replicas, at partitions 0/32/64/96, of
          [-2*Us^T block ; ||Us||^2 row ; ones row]   (K = 18)
      R (114 x m_cols): 4 replicas of [Vs^T ; ones row ; ||Vs||^2 row]
    One augmented matmul then computes sq directly in PSUM; the 4 replicas
    let 4 matmuls run concurrently in the PE's four 32-row groups
    (tile_position row tiling), hiding fp32 LDWEIGHTS + 2-pass cost.
    A single ScalarE activation per [128, 2048] PSUM tile computes
      out = Exp(-0.5 * psum + 2*sn)   (PSUM -> SBUF, bf16 output)
    and the bf16 tiles are DMA'd out in 1 MB chunks; the host upcasts to
    fp32 (bf16 rounding is ~2e-3 relative, inside the 2e-2 gate).

Per core the bf16 output is 16 MB (~45 us at 358 GB/s HBM write); the
8.4M-element Exp on ScalarE (~1 elem/cycle/lane @ 1.2 GHz => ~60 us) is
the steady-state bottleneck, with PE and DMA hidden under it.
"""

import numpy as np

import concourse.bacc as bacc
import concourse.bass as bass  # noqa: F401  (AP helpers)
import concourse.mybir as mybir
import concourse.tile as tile

N, M, D = 8192, 8192, 16
N_CORES = 8
ROWS = N // N_CORES  # 1024 rows of U per core
P = 128              # output partitions per row block
FREE = 512           # matmul moving free dim (fp32 max)
QUAD = 2048          # one ACT instruction: 4 PSUM banks of f32
HALF = 4096          # one output store: [128, 4096] fp16 = 1 MB
K = D + 2            # augmented contraction dim
GAP = 32             # partition stride between the 4 operand replicas
AUG = 3 * GAP + K    # 114 partitions holding the replicated operands

F32 = mybir.dt.float32
F16 = mybir.dt.bfloat16  # fp16 ACT output crashed the exec unit on HW; bf16 is native
AF = mybir.ActivationFunctionType


def build_program(rows=ROWS, m_cols=M, repeats=1):
    """Build the per-core Bass program. rows/m_cols shrinkable for sim."""
    rb = rows // P
    nq = max(1, m_cols // QUAD)

    nc = bacc.Bacc()
    l = nc.declare_dram_parameter("l", [AUG, rows], F32, isOutput=False)
    r = nc.declare_dram_parameter("r", [AUG, m_cols], F32, isOutput=False)
    b = nc.declare_dram_parameter("b", [P, 1], F32, isOutput=False)
    out = nc.declare_dram_parameter("out", [rows, m_cols], F16, isOutput=True)

    with tile.TileContext(nc) as tc:
        with (
            tc.tile_pool(name="singles", bufs=1) as singles,
            tc.tile_pool(name="psum_pool", bufs=2, space="PSUM") as psum_pool,
            tc.tile_pool(name="obuf_pool", bufs=4) as obuf_pool,
        ):
            bt = singles.tile([P, 1], F32)
            nc.sync.dma_start(bt[:], b[:])
            Lt = singles.tile([AUG, rows], F32)
            nc.sync.dma_start(Lt[:], l[:])
            Rt = singles.tile([AUG, m_cols], F32)
            # Load R's first 2048 columns separately so the first matmuls
            # only wait on ~1 MB of the 3.7 MB operand.
            c0 = min(QUAD, m_cols)
            nc.sync.dma_start(Rt[:, 0:c0], r[:, 0:c0])
            if m_cols > c0:
                nc.sync.dma_start(Rt[:, c0:], r[:, c0:])

            for _rep in range(repeats):
                for m in range(rb):
                    for h in range(m_cols // HALF if m_cols >= HALF else 1):
                        hw_ = min(HALF, m_cols)
                        ob = obuf_pool.tile([P, hw_], F16, tag="ob", name="ob")
                        for qq in range(hw_ // QUAD if hw_ >= QUAD else 1):
                            q = h * (HALF // QUAD) + qq
                            qw = min(QUAD, hw_)
                            ps = psum_pool.tile([P, qw], F32, tag="ps", name="ps")
                            for k in range(qw // FREE):
                                n = q * (QUAD // FREE) + k
                                nc.tensor.matmul(
                                    ps[:, k * FREE : (k + 1) * FREE],
                                    Lt[GAP * k : GAP * k + K, m * P : (m + 1) * P],
                                    Rt[GAP * k : GAP * k + K,
                                       n * FREE : (n + 1) * FREE],
                                    start=True, stop=True,
                                    tile_position=(GAP * k, 0),
                                )
                            nc.scalar.activation(
                                ob[:, qq * QUAD : qq * QUAD + qw], ps[:],
                                AF.Exp, bias=bt[:], scale=-0.5,
                            )
                        nc.sync.dma_start(
                            out[m * P : (m + 1) * P, h * HALF : h * HALF + hw_],
                            ob[:],
                        )

    nc.compile()  # bacc lowering: splits multi-waits, reg alloc, etc.
    return nc


_PROGRAM_CACHE = {}


def get_program(rows=ROWS, m_cols=M, repeats=1):
    key = (rows, m_cols, repeats)
    if key not in _PROGRAM_CACHE:
        _PROGRAM_CACHE[key] = build_program(rows, m_cols, repeats)
    return _PROGRAM_CACHE[key]


def make_in_maps(U, V, weights, sn):
    U = np.asarray(U, dtype=np.float32)
    V = np.asarray(V, dtype=np.float32)
    w = np.asarray(weights, dtype=np.float32).reshape(D)
    snf = float(np.asarray(sn, dtype=np.float32).reshape(()))

    s = np.exp(-w.astype(np.float64))
    Us = U.astype(np.float64) * s
    Vs = V.astype(np.float64) * s
    u2 = np.sum(Us * Us, axis=1)                     # (N,)
    v2 = np.sum(Vs * Vs, axis=1)                     # (M,)

    r_small = np.empty((K, M), dtype=np.float32)
    r_small[0:D] = Vs.T
    r_small[D] = 1.0
    r_small[D + 1] = v2
    r_full = np.zeros((AUG, M), dtype=np.float32)
    for g in range(4):
        r_full[GAP * g : GAP * g + K] = r_small
    r_full = np.ascontiguousarray(r_full)

    bias = np.full((P, 1), 2.0 * snf, dtype=np.float32)

    in_maps = []
    for c in range(N_CORES):
        rs = slice(c * ROWS, (c + 1) * ROWS)
        l_small = np.empty((K, ROWS), dtype=np.float32)
        l_small[0:D] = -2.0 * Us[rs].T
        l_small[D] = u2[rs]
        l_small[D + 1] = 1.0
        l_full = np.zeros((AUG, ROWS), dtype=np.float32)
        for g in range(4):
            l_full[GAP * g : GAP * g + K] = l_small
        in_maps.append({
            "l": np.ascontiguousarray(l_full),
            "r": r_full,
            "b": bias,
        })
    return in_maps


def kernel(U, V, weights, sn):
    from concourse.bass_utils import run_bass_kernel_spmd

    nc = get_program()
    in_maps = make_in_maps(U, V, weights, sn)
    res = run_bass_kernel_spmd(nc, in_maps, core_ids=list(range(N_CORES)))
    return np.concatenate(
        [np.asarray(r["out"]).astype(np.float32) for r in res.results], axis=0
    )
